# revision 9
# baseline (speedup 1.0000x reference)
"""AxisAttention TRN2 kernel: 8-core data-parallel over batch b.

Per core: x (256,128,128) fp32. axis='h' attention: 128 sequences (one per w)
of length 128 (h), 256 channels, HEADS=4, head_dim=64.

Stages per sequence w (PE dtypes chosen per stage):
  qkv:  q,k = Wqkv[:, :512].T @ X[:, (w, h)]      f32r, N=256 moving
        (k bias dropped: softmax is invariant to per-query constants)
  vT:   v^T = X_w.T @ [Wv | 0] + [bv | 1]          f32r, N=260 moving
        evac'd into fp16 interleaved layout [s_k, (head, 64 v | 1 one)]
  QK^T: S^T[s_k, s_q] per head                     fp16, K=128 via
        zero-padded block-diag k (all operands at partition offset 0 --
        mixing partition offsets across matmuls faults the PE)
  E^T = exp(S^T / 8)  (ACT -> fp16; no max-subtraction, logits are small)
  AV:   O_w[s_q, 65] = E_h^T.T @ vT_h              fp16, N=65 per head;
        col 64 = softmax denominator (ones column trick)
  norm: one batched reciprocal + per-head tensor_scalar (DVE) -> f32r
  transpose via PE (f32r identity), evac, proj f32r N=256 + bproj.
Output staged in SBUF [co, (h, w-quarter)], DMA'd per quarter.
"""
import sys
sys.path.insert(0, '/opt/trn_rl_repo')
from contextlib import ExitStack

import numpy as np

import concourse.bass as bass
import concourse.tile as tile
from concourse import bacc, mybir
from concourse.bass_utils import run_bass_kernel_spmd
from concourse.masks import make_identity

dt = mybir.dt
AF = mybir.ActivationFunctionType

B, C, H, W = 8, 256, 128, 128
HEADS, HD = 4, 64
SCALE = float(HD) ** -0.5
N_CORES = 8


def build(reps: int = 1):
    nc = bacc.Bacc("TRN2", target_bir_lowering=False, debug=False,
                   num_devices=N_CORES)
    x_d = nc.dram_tensor("x", [C, H, W], dt.float32r, kind="ExternalInput").ap()
    wqkv_d = nc.dram_tensor("Wqkv", [C, 3 * C], dt.float32r, kind="ExternalInput").ap()
    bqkv_d = nc.dram_tensor("bqkv", [3 * C], dt.float32, kind="ExternalInput").ap()
    wproj_d = nc.dram_tensor("Wproj", [C, C], dt.float32r, kind="ExternalInput").ap()
    bproj_d = nc.dram_tensor("bproj", [C], dt.float32, kind="ExternalInput").ap()
    out_d = nc.dram_tensor("out", [C, H, W], dt.float32, kind="ExternalOutput").ap()

    with tile.TileContext(nc) as tc, ExitStack() as ctx:
        const = ctx.enter_context(tc.tile_pool(name="const", bufs=1))
        xp = ctx.enter_context(tc.tile_pool(name="xp", bufs=1))
        stp = ctx.enter_context(tc.tile_pool(name="stp", bufs=1))
        qkp = ctx.enter_context(tc.tile_pool(name="qkp", bufs=2))
        vtp = ctx.enter_context(tc.tile_pool(name="vtp", bufs=8))
        etp = ctx.enter_context(tc.tile_pool(name="etp", bufs=2))
        onp = ctx.enter_context(tc.tile_pool(name="onp", bufs=2))
        otp = ctx.enter_context(tc.tile_pool(name="otp", bufs=2))
        recp = ctx.enter_context(tc.tile_pool(name="recp", bufs=8))
        psA = ctx.enter_context(tc.tile_pool(name="psA", bufs=2, space="PSUM"))
        psS = ctx.enter_context(tc.tile_pool(name="psS", bufs=2, space="PSUM"))
        psV = ctx.enter_context(tc.tile_pool(name="psV", bufs=2, space="PSUM"))
        psO = ctx.enter_context(tc.tile_pool(name="psO", bufs=1, space="PSUM"))
        psAV = ctx.enter_context(tc.tile_pool(name="psAV", bufs=1, space="PSUM"))

        # ---- constants ----
        wqk = []
        wv_aug = []
        wproj = []
        zeros4 = const.tile([128, 4], dt.float32)
        nc.vector.memset(zeros4[:], 0.0)
        for kc in range(2):
            wq_t = const.tile([128, 512], dt.float32r, tag=f"wqk{kc}")
            nc.sync.dma_start(wq_t[:], wqkv_d[kc * 128:(kc + 1) * 128, 0:512])
            wqk.append(wq_t)
            wv_t = const.tile([128, 260], dt.float32r, tag=f"wv{kc}")
            nc.vector.tensor_copy(wv_t[:, 256:260], zeros4[:])
            nc.sync.dma_start(wv_t[:, 0:256], wqkv_d[kc * 128:(kc + 1) * 128, 512:768])
            wv_aug.append(wv_t)
            wp_t = const.tile([128, 256], dt.float32r, tag=f"wp{kc}")
            nc.sync.dma_start(wp_t[:], wproj_d[kc * 128:(kc + 1) * 128, :])
            wproj.append(wp_t)

        bias_qk = const.tile([128, 4], dt.float32)
        nc.sync.dma_start(bias_qk[:], bqkv_d[0:512].rearrange("(j p) -> p j", p=128))
        bias_proj = const.tile([128, 2], dt.float32)
        nc.sync.dma_start(bias_proj[:], bproj_d.rearrange("(j p) -> p j", p=128))

        bv_row = const.tile([1, 256], dt.float32)
        nc.sync.dma_start(bv_row[:], bqkv_d[512:768].rearrange("(o c) -> o c", o=1))
        bv_aug = const.tile([128, 260], dt.float32)
        nc.vector.memset(bv_aug[:, 256:260], 1.0)
        nc.gpsimd.partition_broadcast(bv_aug[:, 0:256], bv_row[:])

        ident = const.tile([128, 128], dt.float32)
        make_identity(nc, ident[:])
        ident_h = const.tile([128, 128], dt.float16)
        nc.vector.tensor_copy(ident_h[:], ident[:])

        # ---- X resident (raw fp32 bits consumed as f32r) ----
        xv = []
        for kc in range(2):
            x_t = xp.tile([128, H * W], dt.float32r, tag=f"x{kc}")
            nc.sync.dma_start(x_t[:], x_d[kc * 128:(kc + 1) * 128, :, :]
                              .rearrange("p h w -> p (h w)"))
            # view [p, w, h]: token (w, h) at free h*W + w
            xv.append(x_t[:].rearrange("p (h w) -> p w h", w=W))

        def emit_qkv(w0):
            # q: [128, (cb 2) x (wloc 4) x 128] fp16
            q_sb = qkp.tile([128, 1024], dt.float16, tag="qk")
            # k: per c-chunk, fp16 zero-padded block-diag:
            #   kz[cb][0:64, 0:512] = k of even head, kz[cb][64:128, 512:1024]
            #   = k of odd head; rest zeros -> K=128 QK matmuls, offset 0.
            kz0 = qkp.tile([128, 1024], dt.float16, tag="kz0")
            kz1 = qkp.tile([128, 1024], dt.float16, tag="kz1")
            kzs = (kz0, kz1)
            for cb in range(4):
                pq = psA.tile([128, 512], dt.float32, tag="mm")
                for kc in range(2):
                    nc.tensor.matmul(pq[:], wqk[kc][:, cb * 128:(cb + 1) * 128],
                                     xv[kc][:, w0:w0 + 4, :],
                                     start=(kc == 0), stop=(kc == 1))
                if cb < 2:
                    nc.scalar.activation(q_sb[:, cb * 512:(cb + 1) * 512], pq[:],
                                         AF.Identity, bias=bias_qk[:, cb:cb + 1])
                else:
                    kz = kzs[cb - 2]
                    nc.vector.memset(kz[64:128, 0:512], 0.0)
                    nc.vector.memset(kz[0:64, 512:1024], 0.0)
                    nc.vector.tensor_copy(kz[0:64, 0:512], pq[0:64, :])
                    nc.vector.tensor_copy(kz[64:128, 512:1024], pq[64:128, :])
            vts = []
            for wloc in range(4):
                pv = psV.tile([128, 260], dt.float32, tag="vt")
                for kc in range(2):
                    nc.tensor.matmul(pv[:], xv[kc][:, w0 + wloc, :], wv_aug[kc][:],
                                     start=(kc == 0), stop=(kc == 1))
                # interleave to [s_k, (head: 64 v, 1 one)] fp16 with +bias
                vt_sb = vtp.tile([128, 260], dt.float16, tag="vts")
                vt_v = vt_sb[:].rearrange("p (h u) -> p h u", u=65)
                pv4 = pv[:, 0:256].rearrange("p (h u) -> p h u", u=64)
                bv4 = bv_aug[:, 0:256].rearrange("p (h u) -> p h u", u=64)
                nc.vector.tensor_add(vt_v[:, :, 0:64], pv4, bv4)
                nc.vector.tensor_add(vt_v[:, :, 64], pv[:, 256:260],
                                     bv_aug[:, 256:260])
                vts.append(vt_sb)
            return (q_sb, kz0, kz1), vts

        def emit_attn(qk_sb, vts, blk, stages):
            q_sb, kz0, kz1 = qk_sb
            kzs = (kz0, kz1)
            ot_sb = otp.tile([128, 1024], dt.float32r, tag="ot")
            for wloc in range(4):
                psc = psS.tile([128, 512], dt.float32, tag="sc")
                for h in range(4):
                    kz = kzs[h // 2]
                    ck = (h % 2) * 512 + wloc * 128
                    cq = (h // 2) * 512 + wloc * 128
                    nc.tensor.matmul(psc[:, h * 128:(h + 1) * 128],
                                     kz[:, ck:ck + 128],
                                     q_sb[:, cq:cq + 128],
                                     start=True, stop=True)
                et = etp.tile([128, 512], dt.float16, tag="et")
                nc.scalar.activation(et[:], psc[:], AF.Exp, scale=SCALE)
                pav = psAV.tile([128, 260], dt.float32, tag="av")
                for h in range(4):
                    nc.tensor.matmul(pav[:, h * 65:(h + 1) * 65],
                                     et[:, h * 128:(h + 1) * 128],
                                     vts[wloc][:, h * 65:(h + 1) * 65],
                                     start=True, stop=True)
                pav_v = pav[:].rearrange("p (h u) -> p h u", u=65)
                rec4 = recp.tile([128, 4], dt.float32, tag="rec")
                nc.vector.reciprocal(rec4[:], pav_v[:, :, 64])
                onorm = onp.tile([128, 256], dt.float16, tag="on")
                for h in range(4):
                    nc.vector.tensor_scalar_mul(onorm[:, h * 64:(h + 1) * 64],
                                                pav_v[:, h, 0:64],
                                                rec4[:, h:h + 1])
                pot = psO.tile([128, 256], dt.float16, tag="pot")
                for kc in range(2):
                    nc.tensor.transpose(pot[:, kc * 128:(kc + 1) * 128],
                                        onorm[:, kc * 128:(kc + 1) * 128],
                                        ident_h[:])
                dst = ot_sb[:].rearrange("p (kc w s) -> p w kc s",
                                         kc=2, w=4)[:, wloc, :, :]
                nc.scalar.copy(dst, pot[:].rearrange("p (kc s) -> p kc s", kc=2))
            for co in range(2):
                pp = psA.tile([128, 512], dt.float32, tag="mm")
                for kc in range(2):
                    nc.tensor.matmul(pp[:], wproj[kc][:, co * 128:(co + 1) * 128],
                                     ot_sb[:, kc * 512:(kc + 1) * 512],
                                     start=(kc == 0), stop=(kc == 1))
                dstv = stages[co][:].rearrange("p (h b wl) -> p b wl h",
                                               b=8, wl=4)[:, blk, :, :]
                nc.scalar.activation(dstv, pp[:].rearrange("p (wl s) -> p wl s", wl=4),
                                     AF.Identity, bias=bias_proj[:, co:co + 1])

        for rep in range(reps):
            cur = emit_qkv(0)
            for wq in range(4):
                stage0 = stp.tile([128, 128 * 32], dt.float32, tag="st0")
                stage1 = stp.tile([128, 128 * 32], dt.float32, tag="st1")
                stages = (stage0, stage1)
                for blk in range(8):
                    nxt_w0 = wq * 32 + blk * 4 + 4
                    nxt = emit_qkv(nxt_w0) if nxt_w0 < W else None
                    emit_attn(cur[0], cur[1], blk, stages)
                    if nxt is not None:
                        cur = nxt
                    elif rep + 1 < reps:
                        cur = emit_qkv(0)
                for co in range(2):
                    dv = out_d[co * 128:(co + 1) * 128, :, wq * 32:(wq + 1) * 32]
                    nc.sync.dma_start(dv, stages[co][:]
                                      .rearrange("p (h w) -> p h w", w=32))

    nc.compile()
    return nc


_NC_CACHE = {}


def _get_nc(reps=1):
    if reps not in _NC_CACHE:
        _NC_CACHE[reps] = build(reps)
    return _NC_CACHE[reps]


def run_on_cores(inputs, reps=1):
    nc = _get_nc(reps)
    x = np.ascontiguousarray(np.asarray(inputs["x"], np.float32))
    base = {
        "Wqkv": np.ascontiguousarray(np.asarray(inputs["Wqkv"], np.float32)),
        "bqkv": np.ascontiguousarray(np.asarray(inputs["bqkv"], np.float32)),
        "Wproj": np.ascontiguousarray(np.asarray(inputs["Wproj"], np.float32)),
        "bproj": np.ascontiguousarray(np.asarray(inputs["bproj"], np.float32)),
    }
    in_maps = [dict(base, x=np.ascontiguousarray(x[i])) for i in range(N_CORES)]
    res = run_bass_kernel_spmd(nc, in_maps, core_ids=list(range(N_CORES)))
    return np.stack([res.results[i]["out"] for i in range(N_CORES)], axis=0)


def kernel(x, Wqkv, bqkv, Wproj, bproj):
    return run_on_cores(
        {"x": x, "Wqkv": Wqkv, "bqkv": bqkv, "Wproj": Wproj, "bproj": bproj})


if __name__ == "__main__":
    np.random.seed(0)
    ins = {
        "x": np.random.randn(B, C, H, W).astype(np.float32),
        "Wqkv": (np.random.randn(C, 3 * C) / 16).astype(np.float32),
        "bqkv": (np.random.randn(3 * C) * 0.02).astype(np.float32),
        "Wproj": (np.random.randn(C, C) / 16).astype(np.float32),
        "bproj": (np.random.randn(C) * 0.02).astype(np.float32),
    }
    out = kernel(**ins)
    print("out", out.shape, out.dtype, float(np.abs(out).max()))


# revision 10
# speedup vs baseline: 1.1242x; 1.1242x over previous
"""AxisAttention TRN2 kernel: 8-core data-parallel over batch b.

Per core: x (256,128,128) fp32. axis='h' attention: 128 sequences (one per w)
of length 128 (h), 256 channels, HEADS=4, head_dim=64.

Stages per sequence w (PE dtypes chosen per stage):
  qkv:  q,k = Wqkv[:, :512].T @ X[:, (w, h)]      f32r, N=256 moving
        (k bias dropped: softmax is invariant to per-query constants)
  vT:   v^T = X_w.T @ [Wv | 0] + [bv | 1]          f32r, N=260 moving
        evac'd into fp16 interleaved layout [s_k, (head, 64 v | 1 one)]
  QK^T: S^T[s_k, s_q] per head                     fp16, K=128 via
        zero-padded block-diag k (all operands at partition offset 0 --
        mixing partition offsets across matmuls faults the PE)
  E^T = exp(S^T / 8)  (ACT -> fp16; no max-subtraction, logits are small)
  AV:   O_w[s_q, 65] = E_h^T.T @ vT_h              fp16, N=65 per head;
        col 64 = softmax denominator (ones column trick)
  norm: one batched reciprocal + per-head tensor_scalar (DVE) -> f32r
  transpose via PE (f32r identity), evac, proj f32r N=256 + bproj.
Output staged in SBUF [co, (h, w-quarter)], DMA'd per quarter.
"""
import sys
sys.path.insert(0, '/opt/trn_rl_repo')
from contextlib import ExitStack

import numpy as np

import concourse.bass as bass
import concourse.tile as tile
from concourse import bacc, mybir
from concourse.bass_utils import run_bass_kernel_spmd
from concourse.masks import make_identity

dt = mybir.dt
AF = mybir.ActivationFunctionType

B, C, H, W = 8, 256, 128, 128
HEADS, HD = 4, 64
SCALE = float(HD) ** -0.5
N_CORES = 8


def build(reps: int = 1):
    nc = bacc.Bacc("TRN2", target_bir_lowering=False, debug=False,
                   num_devices=N_CORES)
    x_d = nc.dram_tensor("x", [C, H, W], dt.float32r, kind="ExternalInput").ap()
    wqkv_d = nc.dram_tensor("Wqkv", [C, 3 * C], dt.float32r, kind="ExternalInput").ap()
    bqkv_d = nc.dram_tensor("bqkv", [3 * C], dt.float32, kind="ExternalInput").ap()
    wproj_d = nc.dram_tensor("Wproj", [C, C], dt.float32r, kind="ExternalInput").ap()
    bproj_d = nc.dram_tensor("bproj", [C], dt.float32, kind="ExternalInput").ap()
    out_d = nc.dram_tensor("out", [C, H, W], dt.float32, kind="ExternalOutput").ap()

    with tile.TileContext(nc) as tc, ExitStack() as ctx:
        const = ctx.enter_context(tc.tile_pool(name="const", bufs=1))
        xp = ctx.enter_context(tc.tile_pool(name="xp", bufs=1))
        stp = ctx.enter_context(tc.tile_pool(name="stp", bufs=1))
        qkp = ctx.enter_context(tc.tile_pool(name="qkp", bufs=2))
        vtp = ctx.enter_context(tc.tile_pool(name="vtp", bufs=8))
        etp = ctx.enter_context(tc.tile_pool(name="etp", bufs=2))
        onp = ctx.enter_context(tc.tile_pool(name="onp", bufs=2))
        otp = ctx.enter_context(tc.tile_pool(name="otp", bufs=2))
        recp = ctx.enter_context(tc.tile_pool(name="recp", bufs=8))
        psA = ctx.enter_context(tc.tile_pool(name="psA", bufs=2, space="PSUM"))
        psS = ctx.enter_context(tc.tile_pool(name="psS", bufs=2, space="PSUM"))
        psV = ctx.enter_context(tc.tile_pool(name="psV", bufs=1, space="PSUM"))
        psO = ctx.enter_context(tc.tile_pool(name="psO", bufs=1, space="PSUM"))
        psAV = ctx.enter_context(tc.tile_pool(name="psAV", bufs=2, space="PSUM"))

        # ---- constants ----
        wqk = []
        wv_aug = []
        wproj = []
        zeros4 = const.tile([128, 4], dt.float32)
        nc.vector.memset(zeros4[:], 0.0)
        for kc in range(2):
            wq_t = const.tile([128, 512], dt.float32r, tag=f"wqk{kc}")
            nc.sync.dma_start(wq_t[:], wqkv_d[kc * 128:(kc + 1) * 128, 0:512])
            wqk.append(wq_t)
            wv_t = const.tile([128, 260], dt.float32r, tag=f"wv{kc}")
            nc.vector.tensor_copy(wv_t[:, 256:260], zeros4[:])
            nc.sync.dma_start(wv_t[:, 0:256], wqkv_d[kc * 128:(kc + 1) * 128, 512:768])
            wv_aug.append(wv_t)
            wp_t = const.tile([128, 256], dt.float32r, tag=f"wp{kc}")
            nc.sync.dma_start(wp_t[:], wproj_d[kc * 128:(kc + 1) * 128, :])
            wproj.append(wp_t)

        bias_qk = const.tile([128, 4], dt.float32)
        nc.sync.dma_start(bias_qk[:], bqkv_d[0:512].rearrange("(j p) -> p j", p=128))
        bias_proj = const.tile([128, 2], dt.float32)
        nc.sync.dma_start(bias_proj[:], bproj_d.rearrange("(j p) -> p j", p=128))

        bv_row = const.tile([1, 256], dt.float32)
        nc.sync.dma_start(bv_row[:], bqkv_d[512:768].rearrange("(o c) -> o c", o=1))
        bv_aug = const.tile([128, 260], dt.float32)
        nc.vector.memset(bv_aug[:, 256:260], 1.0)
        nc.gpsimd.partition_broadcast(bv_aug[:, 0:256], bv_row[:])

        ident = const.tile([128, 128], dt.float32)
        make_identity(nc, ident[:])
        ident_h = const.tile([128, 128], dt.float16)
        nc.vector.tensor_copy(ident_h[:], ident[:])

        # ---- X resident (raw fp32 bits consumed as f32r) ----
        xv = []
        for kc in range(2):
            x_t = xp.tile([128, H * W], dt.float32r, tag=f"x{kc}")
            nc.sync.dma_start(x_t[:], x_d[kc * 128:(kc + 1) * 128, :, :]
                              .rearrange("p h w -> p (h w)"))
            # view [p, w, h]: token (w, h) at free h*W + w
            xv.append(x_t[:].rearrange("p (h w) -> p w h", w=W))

        def emit_qkv(w0):
            # q: [128, (cb 2) x (wloc 4) x 128] fp16
            q_sb = qkp.tile([128, 1024], dt.float16, tag="qk")
            # k: per c-chunk, fp16 zero-padded block-diag:
            #   kz[cb][0:64, 0:512] = k of even head, kz[cb][64:128, 512:1024]
            #   = k of odd head; rest zeros -> K=128 QK matmuls, offset 0.
            kz0 = qkp.tile([128, 1024], dt.float16, tag="kz0")
            kz1 = qkp.tile([128, 1024], dt.float16, tag="kz1")
            kzs = (kz0, kz1)
            for cb in range(4):
                pq = psA.tile([128, 512], dt.float32, tag="mm")
                for kc in range(2):
                    nc.tensor.matmul(pq[:], wqk[kc][:, cb * 128:(cb + 1) * 128],
                                     xv[kc][:, w0:w0 + 4, :],
                                     start=(kc == 0), stop=(kc == 1))
                if cb < 2:
                    nc.scalar.activation(q_sb[:, cb * 512:(cb + 1) * 512], pq[:],
                                         AF.Identity, bias=bias_qk[:, cb:cb + 1])
                else:
                    kz = kzs[cb - 2]
                    nc.vector.memset(kz[64:128, 0:512], 0.0)
                    nc.vector.memset(kz[0:64, 512:1024], 0.0)
                    nc.vector.tensor_copy(kz[0:64, 0:512], pq[0:64, :])
                    nc.vector.tensor_copy(kz[64:128, 512:1024], pq[64:128, :])
            vts = []
            for wloc in range(4):
                pv = psV.tile([128, 260], dt.float32, tag="vt")
                for kc in range(2):
                    nc.tensor.matmul(pv[:], xv[kc][:, w0 + wloc, :], wv_aug[kc][:],
                                     start=(kc == 0), stop=(kc == 1))
                # interleave to [s_k, (head: 64 v, 1 one)] fp16 with +bias
                vt_sb = vtp.tile([128, 260], dt.float16, tag="vts")
                vt_v = vt_sb[:].rearrange("p (h u) -> p h u", u=65)
                pv4 = pv[:, 0:256].rearrange("p (h u) -> p h u", u=64)
                bv4 = bv_aug[:, 0:256].rearrange("p (h u) -> p h u", u=64)
                nc.vector.tensor_add(vt_v[:, :, 0:64], pv4, bv4)
                nc.vector.tensor_add(vt_v[:, :, 64], pv[:, 256:260],
                                     bv_aug[:, 256:260])
                vts.append(vt_sb)
            return (q_sb, kz0, kz1), vts

        def emit_attn(qk_sb, vts, blk, stages):
            q_sb, kz0, kz1 = qk_sb
            kzs = (kz0, kz1)
            ot_sb = otp.tile([128, 1024], dt.float32r, tag="ot")
            for wloc in range(4):
                psc = psS.tile([128, 512], dt.float32, tag="sc")
                for h in range(4):
                    kz = kzs[h // 2]
                    ck = (h % 2) * 512 + wloc * 128
                    cq = (h // 2) * 512 + wloc * 128
                    nc.tensor.matmul(psc[:, h * 128:(h + 1) * 128],
                                     kz[:, ck:ck + 128],
                                     q_sb[:, cq:cq + 128],
                                     start=True, stop=True)
                et = etp.tile([128, 512], dt.float16, tag="et")
                nc.scalar.activation(et[:], psc[:], AF.Exp, scale=SCALE)
                pav = psAV.tile([128, 260], dt.float32, tag="av")
                for h in range(4):
                    nc.tensor.matmul(pav[:, h * 65:(h + 1) * 65],
                                     et[:, h * 128:(h + 1) * 128],
                                     vts[wloc][:, h * 65:(h + 1) * 65],
                                     start=True, stop=True)
                pav_v = pav[:].rearrange("p (h u) -> p h u", u=65)
                rec4 = recp.tile([128, 4], dt.float32, tag="rec")
                nc.vector.reciprocal(rec4[:], pav_v[:, :, 64])
                onorm = onp.tile([128, 256], dt.float16, tag="on")
                for h in range(4):
                    nc.vector.tensor_scalar_mul(onorm[:, h * 64:(h + 1) * 64],
                                                pav_v[:, h, 0:64],
                                                rec4[:, h:h + 1])
                pot = psO.tile([128, 256], dt.float16, tag="pot")
                for kc in range(2):
                    nc.tensor.transpose(pot[:, kc * 128:(kc + 1) * 128],
                                        onorm[:, kc * 128:(kc + 1) * 128],
                                        ident_h[:])
                dst = ot_sb[:].rearrange("p (kc w s) -> p w kc s",
                                         kc=2, w=4)[:, wloc, :, :]
                nc.scalar.copy(dst, pot[:].rearrange("p (kc s) -> p kc s", kc=2))
            for co in range(2):
                pp = psA.tile([128, 512], dt.float32, tag="mm")
                for kc in range(2):
                    nc.tensor.matmul(pp[:], wproj[kc][:, co * 128:(co + 1) * 128],
                                     ot_sb[:, kc * 512:(kc + 1) * 512],
                                     start=(kc == 0), stop=(kc == 1))
                dstv = stages[co][:].rearrange("p (h b wl) -> p b wl h",
                                               b=8, wl=4)[:, blk, :, :]
                nc.scalar.activation(dstv, pp[:].rearrange("p (wl s) -> p wl s", wl=4),
                                     AF.Identity, bias=bias_proj[:, co:co + 1])

        for rep in range(reps):
            cur = emit_qkv(0)
            for wq in range(4):
                stage0 = stp.tile([128, 128 * 32], dt.float32, tag="st0")
                stage1 = stp.tile([128, 128 * 32], dt.float32, tag="st1")
                stages = (stage0, stage1)
                for blk in range(8):
                    nxt_w0 = wq * 32 + blk * 4 + 4
                    nxt = emit_qkv(nxt_w0) if nxt_w0 < W else None
                    emit_attn(cur[0], cur[1], blk, stages)
                    if nxt is not None:
                        cur = nxt
                    elif rep + 1 < reps:
                        cur = emit_qkv(0)
                for co in range(2):
                    dv = out_d[co * 128:(co + 1) * 128, :, wq * 32:(wq + 1) * 32]
                    nc.sync.dma_start(dv, stages[co][:]
                                      .rearrange("p (h w) -> p h w", w=32))

    nc.compile()
    return nc


_NC_CACHE = {}


def _get_nc(reps=1):
    if reps not in _NC_CACHE:
        _NC_CACHE[reps] = build(reps)
    return _NC_CACHE[reps]


def run_on_cores(inputs, reps=1):
    nc = _get_nc(reps)
    x = np.ascontiguousarray(np.asarray(inputs["x"], np.float32))
    base = {
        "Wqkv": np.ascontiguousarray(np.asarray(inputs["Wqkv"], np.float32)),
        "bqkv": np.ascontiguousarray(np.asarray(inputs["bqkv"], np.float32)),
        "Wproj": np.ascontiguousarray(np.asarray(inputs["Wproj"], np.float32)),
        "bproj": np.ascontiguousarray(np.asarray(inputs["bproj"], np.float32)),
    }
    in_maps = [dict(base, x=np.ascontiguousarray(x[i])) for i in range(N_CORES)]
    res = run_bass_kernel_spmd(nc, in_maps, core_ids=list(range(N_CORES)))
    return np.stack([res.results[i]["out"] for i in range(N_CORES)], axis=0)


def kernel(x, Wqkv, bqkv, Wproj, bproj):
    return run_on_cores(
        {"x": x, "Wqkv": Wqkv, "bqkv": bqkv, "Wproj": Wproj, "bproj": bproj})


if __name__ == "__main__":
    np.random.seed(0)
    ins = {
        "x": np.random.randn(B, C, H, W).astype(np.float32),
        "Wqkv": (np.random.randn(C, 3 * C) / 16).astype(np.float32),
        "bqkv": (np.random.randn(3 * C) * 0.02).astype(np.float32),
        "Wproj": (np.random.randn(C, C) / 16).astype(np.float32),
        "bproj": (np.random.randn(C) * 0.02).astype(np.float32),
    }
    out = kernel(**ins)
    print("out", out.shape, out.dtype, float(np.abs(out).max()))
